# revision 2
# baseline (speedup 1.0000x reference)
"""MicrotubuleAttention TRN2 kernel: head-sharded across 8 NeuronCores.

Core c handles q-heads {2c, 2c+1} and kv-head c//2.  Host prepares per-core
weight shards + RoPE tables; each core computes QKV projections, RoPE,
bias-augmented causal attention and its slice of the output projection.
Host sums the 8 partial output projections (the all-reduce).

Math note: within the causal mask the reference bias is
  -(clip(pol)/4096 + softplus-gamma) * (i-j) + gate*sigmoid(A_i.B_j)
The -c*i part is constant per row and cancels in softmax, so the kernel
only adds the per-column term  c*j  plus the low-rank sigmoid term.
"""
import numpy as np

D_MODEL = 2048
N_HEADS = 16
D_HEAD = 128
MAX_SEQ_LEN = 4096
RANK = 32
ROPE_BASE = 10000.0
T = 2048
N_CORES = 8
HPC = N_HEADS // N_CORES          # q heads per core = 2
P = 128
NEG_FILL = -1.0e30


def _build_kernel():
    import concourse.bass as bass
    import concourse.mybir as mybir
    import concourse.tile as tile
    from concourse import bacc
    from concourse.masks import make_identity
    from contextlib import ExitStack

    f32 = mybir.dt.float32
    bf16 = mybir.dt.bfloat16
    AF = mybir.ActivationFunctionType
    ALU = mybir.AluOpType
    AX = mybir.AxisListType

    nc = bacc.Bacc("TRN2", target_bir_lowering=False, debug=False,
                   num_devices=N_CORES)

    xT = nc.dram_tensor("xT", [D_MODEL, T], bf16, kind="ExternalInput")
    wq = nc.dram_tensor("wq", [D_MODEL, HPC * D_HEAD], bf16, kind="ExternalInput")
    wk = nc.dram_tensor("wk", [D_MODEL, D_HEAD], bf16, kind="ExternalInput")
    wv = nc.dram_tensor("wv", [D_MODEL, D_HEAD], bf16, kind="ExternalInput")
    wo = nc.dram_tensor("wo", [HPC * D_HEAD, D_MODEL], bf16, kind="ExternalInput")
    wa = nc.dram_tensor("wa", [D_MODEL, RANK], bf16, kind="ExternalInput")
    wb = nc.dram_tensor("wb", [D_MODEL, RANK], bf16, kind="ExternalInput")
    cosf = nc.dram_tensor("cosf", [T, D_HEAD], f32, kind="ExternalInput")
    sinf = nc.dram_tensor("sinf", [T, D_HEAD], f32, kind="ExternalInput")
    # [1, 4] = [c_h0, c_h1, gate0, gate1]
    hpar = nc.dram_tensor("hpar", [1, 4], f32, kind="ExternalInput")
    out = nc.dram_tensor("out", [T, D_MODEL], f32, kind="ExternalOutput")

    NT = T // P
    ND = D_MODEL // P
    INV_SQRT_D = 1.0 / np.sqrt(float(D_HEAD))
    QKV_W = HPC * D_HEAD + 2 * D_HEAD   # 512: [q0 q1 k v]

    with tile.TileContext(nc) as tc, ExitStack() as ctx:
        singles = ctx.enter_context(tc.tile_pool(name="singles", bufs=1))
        work = ctx.enter_context(tc.tile_pool(name="work", bufs=3))
        scores = ctx.enter_context(tc.tile_pool(name="scores", bufs=2))
        small = ctx.enter_context(tc.tile_pool(name="small", bufs=4))
        psA = ctx.enter_context(tc.tile_pool(name="psA", bufs=2, space="PSUM"))
        psB = ctx.enter_context(tc.tile_pool(name="psB", bufs=2, space="PSUM"))
        psC = ctx.enter_context(tc.tile_pool(name="psC", bufs=2, space="PSUM"))

        ident = singles.tile([P, P], bf16)
        make_identity(nc, ident)

        hbc = singles.tile([P, 4], f32)
        hap = hpar[:]
        nc.sync.dma_start(
            out=hbc[:],
            in_=bass.AP(tensor=hap.tensor, offset=hap.offset,
                        ap=[[0, P], hap.ap[1]]))

        # cj[h][p, j] = c_h * j  (same for every partition p)
        jjf = singles.tile([P, T], f32)
        nc.gpsimd.iota(jjf[:], pattern=[[1, T]], base=0, channel_multiplier=0,
                       allow_small_or_imprecise_dtypes=True)
        cj = singles.tile([P, HPC, T], f32)
        for h in range(HPC):
            nc.vector.tensor_scalar_mul(cj[:, h], jjf[:], hbc[:, h:h + 1])

        qt_sb = singles.tile([P, HPC, T], bf16)      # Q^T per head [d, t]
        kt_sb = singles.tile([P, T], bf16)           # K^T [d, t]
        v_sb = singles.tile([P, NT, D_HEAD], bf16)   # V  [t, d] tiles
        at_sb = singles.tile([P, T], bf16)           # A^T [r, t] (rows 0:32)
        bt_sb = singles.tile([P, T], bf16)
        aoutT = singles.tile([P, HPC, T], bf16)      # attn-out^T [d, t] per head

        # ---------------- projections + RoPE (scoped SBUF) ----------------
        with tc.tile_pool(name="proj", bufs=1) as proj:
            xT_sb = proj.tile([P, ND, T], bf16)
            for d in range(ND):
                nc.sync.dma_start(out=xT_sb[:, d], in_=xT[d * P:(d + 1) * P, :])
            wq_sb = proj.tile([P, ND, HPC * D_HEAD], bf16)
            wk_sb = proj.tile([P, ND, D_HEAD], bf16)
            wv_sb = proj.tile([P, ND, D_HEAD], bf16)
            wa_sb = proj.tile([P, ND, RANK], bf16)
            wb_sb = proj.tile([P, ND, RANK], bf16)
            for d in range(ND):
                sl = slice(d * P, (d + 1) * P)
                nc.sync.dma_start(out=wq_sb[:, d], in_=wq[sl, :])
                nc.sync.dma_start(out=wk_sb[:, d], in_=wk[sl, :])
                nc.sync.dma_start(out=wv_sb[:, d], in_=wv[sl, :])
                nc.sync.dma_start(out=wa_sb[:, d], in_=wa[sl, :])
                nc.sync.dma_start(out=wb_sb[:, d], in_=wb[sl, :])
            cos_sb = proj.tile([P, NT, D_HEAD], f32)
            sin_sb = proj.tile([P, NT, D_HEAD], f32)
            for i in range(NT):
                nc.sync.dma_start(out=cos_sb[:, i], in_=cosf[i * P:(i + 1) * P, :])
                nc.sync.dma_start(out=sin_sb[:, i], in_=sinf[i * P:(i + 1) * P, :])

            for i in range(NT):
                tsl = slice(i * P, (i + 1) * P)
                pq = psA.tile([P, 512], mybir.dt.float32, tag="ps512")
                # sequential accumulation chains: Q (cols 0:256), K, V
                for d in range(ND):
                    nc.tensor.matmul(pq[:, 0:256], xT_sb[:, d, tsl], wq_sb[:, d],
                                     start=(d == 0), stop=(d == ND - 1))
                for d in range(ND):
                    nc.tensor.matmul(pq[:, 256:384], xT_sb[:, d, tsl], wk_sb[:, d],
                                     start=(d == 0), stop=(d == ND - 1))
                for d in range(ND):
                    nc.tensor.matmul(pq[:, 384:512], xT_sb[:, d, tsl], wv_sb[:, d],
                                     start=(d == 0), stop=(d == ND - 1))
                nc.scalar.copy(v_sb[:, i], pq[:, 384:512])
                for hh in range(3):      # q0, q1, k
                    src = pq[:, hh * D_HEAD:(hh + 1) * D_HEAD]
                    rot = work.tile([P, D_HEAD], f32, tag="rot")
                    nc.scalar.mul(rot[:, 0:64], src[:, 64:128], -1.0)
                    nc.scalar.copy(rot[:, 64:128], src[:, 0:64])
                    m1 = work.tile([P, D_HEAD], f32, tag="m1")
                    nc.vector.tensor_mul(m1[:], src, cos_sb[:, i])
                    m2 = work.tile([P, D_HEAD], f32, tag="m2")
                    nc.vector.tensor_mul(m2[:], rot[:], sin_sb[:, i])
                    roped = work.tile([P, D_HEAD], bf16, tag="roped")
                    nc.vector.tensor_add(roped[:], m1[:], m2[:])
                    ptr = psB.tile([P, P], bf16, tag="pt")
                    nc.tensor.transpose(ptr[:], roped[:], ident[:])
                    dst = qt_sb[:, hh, tsl] if hh < HPC else kt_sb[:, tsl]
                    nc.vector.tensor_copy(dst, ptr[:])
                # A^T / B^T in [r, t] layout
                pab = psA.tile([P, 512], mybir.dt.float32, tag="ps512")
                for d in range(ND):
                    nc.tensor.matmul(pab[0:RANK, 0:P], wa_sb[:, d],
                                     xT_sb[:, d, tsl],
                                     start=(d == 0), stop=(d == ND - 1))
                for d in range(ND):
                    nc.tensor.matmul(pab[0:RANK, P:2 * P], wb_sb[:, d],
                                     xT_sb[:, d, tsl],
                                     start=(d == 0), stop=(d == ND - 1))
                nc.scalar.copy(at_sb[0:RANK, tsl], pab[0:RANK, 0:P])
                nc.scalar.copy(bt_sb[0:RANK, tsl], pab[0:RANK, P:2 * P])

        # ---------------- attention ----------------
        for i in range(NT):
            tsl = slice(i * P, (i + 1) * P)
            L = (i + 1) * P
            nchunk = (L + 511) // 512
            # low-rank sigmoid term, shared by both heads
            msig = scores.tile([P, T], f32, tag="msig")
            for cch in range(nchunk):
                c0 = cch * 512
                w = min(512, L - c0)
                mp = psA.tile([P, 512], mybir.dt.float32, tag="ps512")
                nc.tensor.matmul(mp[:, 0:w], at_sb[0:RANK, tsl],
                                 bt_sb[0:RANK, c0:c0 + w])
                nc.scalar.activation(msig[:, c0:c0 + w], mp[:, 0:w], AF.Sigmoid)

            for h in range(HPC):
                sc = scores.tile([P, T], f32, tag="sc")
                for cch in range(nchunk):
                    c0 = cch * 512
                    w = min(512, L - c0)
                    sp = psA.tile([P, 512], mybir.dt.float32, tag="ps512")
                    nc.tensor.matmul(sp[:, 0:w], qt_sb[:, h, tsl],
                                     kt_sb[:, c0:c0 + w])
                    t1 = work.tile([P, 512], f32, tag="t1")
                    nc.vector.scalar_tensor_tensor(
                        t1[:, 0:w], msig[:, c0:c0 + w],
                        hbc[:, HPC + h:HPC + h + 1], cj[:, h, c0:c0 + w],
                        op0=ALU.mult, op1=ALU.add)
                    nc.vector.scalar_tensor_tensor(
                        sc[:, c0:c0 + w], sp[:, 0:w], INV_SQRT_D,
                        t1[:, 0:w], op0=ALU.mult, op1=ALU.add)
                # causal mask on the diagonal 128 block: keep col f <= row p
                nc.gpsimd.affine_select(
                    out=sc[:, L - P:L], in_=sc[:, L - P:L],
                    pattern=[[-1, P]], compare_op=ALU.is_ge,
                    fill=NEG_FILL, base=0, channel_multiplier=1)
                negmax = small.tile([P, 1], f32, tag="negmax")
                nc.vector.tensor_reduce(negmax[:], sc[:, 0:L], axis=AX.X,
                                        op=ALU.max, negate=True)
                rowsum = small.tile([P, 1], f32, tag="rowsum")
                pmat = scores.tile([P, T], bf16, tag="pmat")
                nc.scalar.activation(pmat[:, 0:L], sc[:, 0:L], AF.Exp,
                                     bias=negmax[:], scale=1.0,
                                     accum_out=rowsum[:])
                logsum = small.tile([P, 1], f32, tag="logsum")
                nc.scalar.activation(logsum[:], rowsum[:], AF.Ln)
                bias2 = small.tile([P, 1], f32, tag="bias2")
                nc.vector.tensor_sub(bias2[:], negmax[:], logsum[:])
                nc.scalar.activation(pmat[:, 0:L], sc[:, 0:L], AF.Exp,
                                     bias=bias2[:], scale=1.0)
                ot = psC.tile([P, P], mybir.dt.float32, tag="ot")
                for j in range(i + 1):
                    ptp = psB.tile([P, P], bf16, tag="pt")
                    nc.tensor.transpose(ptp[:], pmat[:, j * P:(j + 1) * P],
                                        ident[:])
                    pts = work.tile([P, P], bf16, tag="pts")
                    nc.vector.tensor_copy(pts[:], ptp[:])
                    nc.tensor.matmul(ot[:], v_sb[:, j], pts[:],
                                     start=(j == 0), stop=(j == i))
                nc.scalar.copy(aoutT[:, h, tsl], ot[:])

        # ---------------- output projection ----------------
        with tc.tile_pool(name="oproj", bufs=1) as oproj, \
             tc.tile_pool(name="outp", bufs=3) as outp:
            wo_sb = oproj.tile([P, HPC, D_MODEL], bf16)
            for h in range(HPC):
                nc.sync.dma_start(out=wo_sb[:, h], in_=wo[h * P:(h + 1) * P, :])
            for i in range(NT):
                tsl = slice(i * P, (i + 1) * P)
                for mch in range(D_MODEL // 512):
                    po = psC.tile([P, 512], mybir.dt.float32, tag="po")
                    for h in range(HPC):
                        nc.tensor.matmul(po[:], aoutT[:, h, tsl],
                                         wo_sb[:, h, mch * 512:(mch + 1) * 512],
                                         start=(h == 0), stop=(h == HPC - 1))
                    ob = outp.tile([P, 512], f32, tag="ob")
                    nc.vector.tensor_copy(ob[:], po[:])
                    nc.sync.dma_start(
                        out=out[tsl, mch * 512:(mch + 1) * 512], in_=ob[:])
    nc.compile()
    return nc


_NC_CACHE = None


def kernel(**inputs):
    global _NC_CACHE
    x = np.asarray(inputs["x"])
    Wq = np.asarray(inputs["Wq"]); Wk = np.asarray(inputs["Wk"])
    Wv = np.asarray(inputs["Wv"]); Wo = np.asarray(inputs["Wo"])
    pol_dir = np.asarray(inputs["pol_dir"]); pol_WA = np.asarray(inputs["pol_WA"])
    pol_WB = np.asarray(inputs["pol_WB"]); pol_gate = np.asarray(inputs["pol_gate"])
    gtp_gamma = np.asarray(inputs["gtp_gamma"])

    import ml_dtypes
    bf = ml_dtypes.bfloat16
    assert x.shape == (1, T, D_MODEL)

    pol = np.clip(pol_dir.astype(np.float64), -1.0, 1.0)
    gamma = np.maximum(np.log1p(np.exp(gtp_gamma.astype(np.float64))), 1e-6)
    c_h = (pol / float(MAX_SEQ_LEN) + gamma).astype(np.float32)
    gate = (1.0 / (1.0 + np.exp(-pol_gate.astype(np.float64)))).astype(np.float32)

    inv_freq = 1.0 / (ROPE_BASE ** (np.arange(0, D_HEAD, 2, dtype=np.float64) / D_HEAD))
    ang = np.arange(T, dtype=np.float64)[:, None] * inv_freq[None, :]
    cosf = np.concatenate([np.cos(ang), np.cos(ang)], 1).astype(np.float32)
    sinf = np.concatenate([np.sin(ang), np.sin(ang)], 1).astype(np.float32)

    xT = np.ascontiguousarray(x[0].T).astype(bf)

    in_maps = []
    for c in range(N_CORES):
        hs = slice(2 * c * D_HEAD, (2 * c + 2) * D_HEAD)
        kvh = c // 2
        in_maps.append({
            "xT": xT,
            "wq": np.ascontiguousarray(Wq[:, hs]).astype(bf),
            "wk": np.ascontiguousarray(Wk[:, kvh * D_HEAD:(kvh + 1) * D_HEAD]).astype(bf),
            "wv": np.ascontiguousarray(Wv[:, kvh * D_HEAD:(kvh + 1) * D_HEAD]).astype(bf),
            "wo": np.ascontiguousarray(Wo[hs, :]).astype(bf),
            "wa": pol_WA.astype(bf),
            "wb": pol_WB.astype(bf),
            "cosf": cosf, "sinf": sinf,
            "hpar": np.array([[c_h[2 * c], c_h[2 * c + 1],
                               gate[2 * c], gate[2 * c + 1]]], dtype=np.float32),
        })

    if _NC_CACHE is None:
        _NC_CACHE = _build_kernel()
    from concourse.bass_utils import run_bass_kernel_spmd
    res = run_bass_kernel_spmd(_NC_CACHE, in_maps, core_ids=list(range(N_CORES)))
    total = np.zeros((T, D_MODEL), dtype=np.float32)
    for c in range(N_CORES):
        total += res.results[c]["out"]
    return total[None, :, :]


# revision 5
# speedup vs baseline: 1.0305x; 1.0305x over previous
"""MicrotubuleAttention TRN2 kernel: head-sharded across 8 NeuronCores.

Core c handles q-heads {2c, 2c+1} and kv-head c//2.  Host prepares per-core
weight shards + RoPE tables; each core computes QKV projections, RoPE,
bias-augmented causal attention and its slice of the output projection.
Host sums the 8 partial output projections (the all-reduce).

Math note: within the causal mask the reference bias is
  -(clip(pol)/4096 + softplus-gamma) * (i-j) + gate*sigmoid(A_i.B_j)
The -c*i part is constant per row and cancels in softmax, so the kernel
only adds the per-column term  c*j  plus the low-rank sigmoid term.
"""
import numpy as np

D_MODEL = 2048
N_HEADS = 16
D_HEAD = 128
MAX_SEQ_LEN = 4096
RANK = 32
ROPE_BASE = 10000.0
T = 2048
N_CORES = 8
HPC = N_HEADS // N_CORES          # q heads per core = 2
P = 128
NEG_FILL = -1.0e30


def _build_kernel():
    import concourse.bass as bass
    import concourse.mybir as mybir
    import concourse.tile as tile
    from concourse import bacc
    from concourse.masks import make_identity
    from contextlib import ExitStack

    f32 = mybir.dt.float32
    bf16 = mybir.dt.bfloat16
    AF = mybir.ActivationFunctionType
    ALU = mybir.AluOpType
    AX = mybir.AxisListType

    nc = bacc.Bacc("TRN2", target_bir_lowering=False, debug=False,
                   num_devices=N_CORES)

    xT = nc.dram_tensor("xT", [D_MODEL, T], bf16, kind="ExternalInput")
    wq = nc.dram_tensor("wq", [D_MODEL, HPC * D_HEAD], bf16, kind="ExternalInput")
    wk = nc.dram_tensor("wk", [D_MODEL, D_HEAD], bf16, kind="ExternalInput")
    wv = nc.dram_tensor("wv", [D_MODEL, D_HEAD], bf16, kind="ExternalInput")
    wo = nc.dram_tensor("wo", [HPC * D_HEAD, D_MODEL], bf16, kind="ExternalInput")
    wa = nc.dram_tensor("wa", [D_MODEL, RANK], bf16, kind="ExternalInput")
    wb = nc.dram_tensor("wb", [D_MODEL, RANK], bf16, kind="ExternalInput")
    cosf = nc.dram_tensor("cosf", [T, D_HEAD], f32, kind="ExternalInput")
    sinf = nc.dram_tensor("sinf", [T, D_HEAD], f32, kind="ExternalInput")
    # [1, 4] = [c_h0, c_h1, gate0, gate1]
    hpar = nc.dram_tensor("hpar", [1, 4], f32, kind="ExternalInput")
    out = nc.dram_tensor("out", [T, D_MODEL], f32, kind="ExternalOutput")

    NT = T // P
    ND = D_MODEL // P
    INV_SQRT_D = 1.0 / np.sqrt(float(D_HEAD))
    QKV_W = HPC * D_HEAD + 2 * D_HEAD   # 512: [q0 q1 k v]

    with tile.TileContext(nc) as tc, ExitStack() as ctx:
        singles = ctx.enter_context(tc.tile_pool(name="singles", bufs=1))
        work = ctx.enter_context(tc.tile_pool(name="work", bufs=3))
        scores = ctx.enter_context(tc.tile_pool(name="scores", bufs=2))
        small = ctx.enter_context(tc.tile_pool(name="small", bufs=4))
        psA = ctx.enter_context(tc.tile_pool(name="psA", bufs=2, space="PSUM"))
        psB = ctx.enter_context(tc.tile_pool(name="psB", bufs=2, space="PSUM"))
        psC = ctx.enter_context(tc.tile_pool(name="psC", bufs=2, space="PSUM"))

        ident = singles.tile([P, P], bf16)
        make_identity(nc, ident)

        hbc = singles.tile([P, 4], f32)
        hap = hpar[:]
        nc.sync.dma_start(
            out=hbc[:],
            in_=bass.AP(tensor=hap.tensor, offset=hap.offset,
                        ap=[[0, P], hap.ap[1]]))

        # cj[h][p, j] = c_h * j  (same for every partition p)
        jjf = singles.tile([P, T], f32)
        nc.gpsimd.iota(jjf[:], pattern=[[1, T]], base=0, channel_multiplier=0,
                       allow_small_or_imprecise_dtypes=True)
        cj = singles.tile([P, HPC, T], f32)
        for h in range(HPC):
            nc.vector.tensor_scalar_mul(cj[:, h], jjf[:], hbc[:, h:h + 1])

        qt_sb = singles.tile([P, HPC, T], bf16)      # Q^T per head [d, t]
        kt_sb = singles.tile([P, T], bf16)           # K^T [d, t]
        v_sb = singles.tile([P, NT, D_HEAD], bf16)   # V  [t, d] tiles
        at_sb = singles.tile([P, T], bf16)           # A^T [r, t] (rows 0:32)
        bt_sb = singles.tile([P, T], bf16)
        aoutT = singles.tile([P, HPC, T], bf16)      # attn-out^T [d, t] per head

        # ---------------- projections + RoPE (scoped SBUF) ----------------
        with tc.tile_pool(name="proj", bufs=1) as proj:
            xT_sb = proj.tile([P, ND, T], bf16)
            for d in range(ND):
                nc.sync.dma_start(out=xT_sb[:, d], in_=xT[d * P:(d + 1) * P, :])
            wq_sb = proj.tile([P, ND, HPC * D_HEAD], bf16)
            wk_sb = proj.tile([P, ND, D_HEAD], bf16)
            wv_sb = proj.tile([P, ND, D_HEAD], bf16)
            wa_sb = proj.tile([P, ND, RANK], bf16)
            wb_sb = proj.tile([P, ND, RANK], bf16)
            for d in range(ND):
                sl = slice(d * P, (d + 1) * P)
                nc.sync.dma_start(out=wq_sb[:, d], in_=wq[sl, :])
                nc.sync.dma_start(out=wk_sb[:, d], in_=wk[sl, :])
                nc.sync.dma_start(out=wv_sb[:, d], in_=wv[sl, :])
                nc.sync.dma_start(out=wa_sb[:, d], in_=wa[sl, :])
                nc.sync.dma_start(out=wb_sb[:, d], in_=wb[sl, :])
            cos_sb = proj.tile([P, NT, D_HEAD], f32)
            sin_sb = proj.tile([P, NT, D_HEAD], f32)
            for i in range(NT):
                nc.sync.dma_start(out=cos_sb[:, i], in_=cosf[i * P:(i + 1) * P, :])
                nc.sync.dma_start(out=sin_sb[:, i], in_=sinf[i * P:(i + 1) * P, :])

            for i in range(NT):
                tsl = slice(i * P, (i + 1) * P)
                pq = psA.tile([P, 512], mybir.dt.float32, tag="ps512")
                # sequential accumulation chains: Q (cols 0:256), K, V
                for d in range(ND):
                    nc.tensor.matmul(pq[:, 0:256], xT_sb[:, d, tsl], wq_sb[:, d],
                                     start=(d == 0), stop=(d == ND - 1))
                for d in range(ND):
                    nc.tensor.matmul(pq[:, 256:384], xT_sb[:, d, tsl], wk_sb[:, d],
                                     start=(d == 0), stop=(d == ND - 1))
                for d in range(ND):
                    nc.tensor.matmul(pq[:, 384:512], xT_sb[:, d, tsl], wv_sb[:, d],
                                     start=(d == 0), stop=(d == ND - 1))
                nc.scalar.copy(v_sb[:, i], pq[:, 384:512])
                for hh in range(3):      # q0, q1, k
                    src = pq[:, hh * D_HEAD:(hh + 1) * D_HEAD]
                    rot = work.tile([P, D_HEAD], f32, tag="rot")
                    nc.scalar.mul(rot[:, 0:64], src[:, 64:128], -1.0)
                    nc.scalar.copy(rot[:, 64:128], src[:, 0:64])
                    m1 = work.tile([P, D_HEAD], f32, tag="m1")
                    nc.vector.tensor_mul(m1[:], src, cos_sb[:, i])
                    m2 = work.tile([P, D_HEAD], f32, tag="m2")
                    nc.vector.tensor_mul(m2[:], rot[:], sin_sb[:, i])
                    roped = work.tile([P, D_HEAD], bf16, tag="roped")
                    nc.vector.tensor_add(roped[:], m1[:], m2[:])
                    ptr = psB.tile([P, P], bf16, tag="pt")
                    nc.tensor.transpose(ptr[:], roped[:], ident[:])
                    dst = qt_sb[:, hh, tsl] if hh < HPC else kt_sb[:, tsl]
                    nc.vector.tensor_copy(dst, ptr[:])
            # A^T / B^T in [r, t] layout, 512-wide moving operand
            for tch in range(T // 512):
                csl = slice(tch * 512, (tch + 1) * 512)
                pab = psA.tile([P, 512], mybir.dt.float32, tag="ps512")
                for d in range(ND):
                    nc.tensor.matmul(pab[0:RANK, :], wa_sb[:, d],
                                     xT_sb[:, d, csl],
                                     start=(d == 0), stop=(d == ND - 1))
                nc.scalar.copy(at_sb[0:RANK, csl], pab[0:RANK, :])
                pbb = psA.tile([P, 512], mybir.dt.float32, tag="ps512")
                for d in range(ND):
                    nc.tensor.matmul(pbb[0:RANK, :], wb_sb[:, d],
                                     xT_sb[:, d, csl],
                                     start=(d == 0), stop=(d == ND - 1))
                nc.scalar.copy(bt_sb[0:RANK, csl], pbb[0:RANK, :])

        # ---------------- attention ----------------
        for i in range(NT):
            tsl = slice(i * P, (i + 1) * P)
            L = (i + 1) * P
            nchunk = (L + 511) // 512
            # low-rank sigmoid term, shared by both heads
            msig = scores.tile([P, T], f32, tag="msig")
            for cch in range(nchunk):
                c0 = cch * 512
                w = min(512, L - c0)
                mp = psA.tile([P, 512], mybir.dt.float32, tag="ps512")
                nc.tensor.matmul(mp[:, 0:w], at_sb[0:RANK, tsl],
                                 bt_sb[0:RANK, c0:c0 + w])
                nc.scalar.activation(msig[:, c0:c0 + w], mp[:, 0:w], AF.Sigmoid)

            for h in range(HPC):
                sc = scores.tile([P, T], f32, tag="sc")
                for cch in range(nchunk):
                    c0 = cch * 512
                    w = min(512, L - c0)
                    sp = psA.tile([P, 512], mybir.dt.float32, tag="ps512")
                    nc.tensor.matmul(sp[:, 0:w], qt_sb[:, h, tsl],
                                     kt_sb[:, c0:c0 + w])
                    t1 = work.tile([P, 512], f32, tag="t1")
                    nc.vector.scalar_tensor_tensor(
                        t1[:, 0:w], msig[:, c0:c0 + w],
                        hbc[:, HPC + h:HPC + h + 1], cj[:, h, c0:c0 + w],
                        op0=ALU.mult, op1=ALU.add)
                    nc.vector.scalar_tensor_tensor(
                        sc[:, c0:c0 + w], sp[:, 0:w], INV_SQRT_D,
                        t1[:, 0:w], op0=ALU.mult, op1=ALU.add)
                # causal mask on the diagonal 128 block: keep col f <= row p
                nc.gpsimd.affine_select(
                    out=sc[:, L - P:L], in_=sc[:, L - P:L],
                    pattern=[[-1, P]], compare_op=ALU.is_ge,
                    fill=NEG_FILL, base=0, channel_multiplier=1)
                negmax = small.tile([P, 1], f32, tag="negmax")
                nc.vector.tensor_reduce(negmax[:], sc[:, 0:L], axis=AX.X,
                                        op=ALU.max, negate=True)
                rowsum = small.tile([P, 1], f32, tag="rowsum")
                pmat = scores.tile([P, T], bf16, tag="pmat")
                nc.scalar.activation(pmat[:, 0:L], sc[:, 0:L], AF.Exp,
                                     bias=negmax[:], scale=1.0,
                                     accum_out=rowsum[:])
                logsum = small.tile([P, 1], f32, tag="logsum")
                nc.scalar.activation(logsum[:], rowsum[:], AF.Ln)
                bias2 = small.tile([P, 1], f32, tag="bias2")
                nc.vector.tensor_sub(bias2[:], negmax[:], logsum[:])
                nc.scalar.activation(pmat[:, 0:L], sc[:, 0:L], AF.Exp,
                                     bias=bias2[:], scale=1.0)
                ot = psC.tile([P, P], mybir.dt.float32, tag="ot")
                for j in range(i + 1):
                    ptp = psB.tile([P, P], bf16, tag="pt")
                    nc.tensor.transpose(ptp[:], pmat[:, j * P:(j + 1) * P],
                                        ident[:])
                    pts = work.tile([P, P], bf16, tag="pts")
                    nc.vector.tensor_copy(pts[:], ptp[:])
                    nc.tensor.matmul(ot[:], v_sb[:, j], pts[:],
                                     start=(j == 0), stop=(j == i))
                nc.scalar.copy(aoutT[:, h, tsl], ot[:])

        # ---------------- output projection ----------------
        with tc.tile_pool(name="oproj", bufs=1) as oproj, \
             tc.tile_pool(name="outp", bufs=3) as outp:
            wo_sb = oproj.tile([P, HPC, D_MODEL], bf16)
            for h in range(HPC):
                nc.sync.dma_start(out=wo_sb[:, h], in_=wo[h * P:(h + 1) * P, :])
            for i in range(NT):
                tsl = slice(i * P, (i + 1) * P)
                for mch in range(D_MODEL // 512):
                    po = psC.tile([P, 512], mybir.dt.float32, tag="po")
                    for h in range(HPC):
                        nc.tensor.matmul(po[:], aoutT[:, h, tsl],
                                         wo_sb[:, h, mch * 512:(mch + 1) * 512],
                                         start=(h == 0), stop=(h == HPC - 1))
                    ob = outp.tile([P, 512], f32, tag="ob")
                    nc.vector.tensor_copy(ob[:], po[:])
                    nc.sync.dma_start(
                        out=out[tsl, mch * 512:(mch + 1) * 512], in_=ob[:])
    nc.compile()
    return nc


_NC_CACHE = None


def kernel(**inputs):
    global _NC_CACHE
    x = np.asarray(inputs["x"])
    Wq = np.asarray(inputs["Wq"]); Wk = np.asarray(inputs["Wk"])
    Wv = np.asarray(inputs["Wv"]); Wo = np.asarray(inputs["Wo"])
    pol_dir = np.asarray(inputs["pol_dir"]); pol_WA = np.asarray(inputs["pol_WA"])
    pol_WB = np.asarray(inputs["pol_WB"]); pol_gate = np.asarray(inputs["pol_gate"])
    gtp_gamma = np.asarray(inputs["gtp_gamma"])

    import ml_dtypes
    bf = ml_dtypes.bfloat16
    assert x.shape == (1, T, D_MODEL)

    pol = np.clip(pol_dir.astype(np.float64), -1.0, 1.0)
    gamma = np.maximum(np.log1p(np.exp(gtp_gamma.astype(np.float64))), 1e-6)
    c_h = (pol / float(MAX_SEQ_LEN) + gamma).astype(np.float32)
    gate = (1.0 / (1.0 + np.exp(-pol_gate.astype(np.float64)))).astype(np.float32)

    inv_freq = 1.0 / (ROPE_BASE ** (np.arange(0, D_HEAD, 2, dtype=np.float64) / D_HEAD))
    ang = np.arange(T, dtype=np.float64)[:, None] * inv_freq[None, :]
    cosf = np.concatenate([np.cos(ang), np.cos(ang)], 1).astype(np.float32)
    sinf = np.concatenate([np.sin(ang), np.sin(ang)], 1).astype(np.float32)

    xT = np.ascontiguousarray(x[0].T).astype(bf)

    in_maps = []
    for c in range(N_CORES):
        hs = slice(2 * c * D_HEAD, (2 * c + 2) * D_HEAD)
        kvh = c // 2
        in_maps.append({
            "xT": xT,
            "wq": np.ascontiguousarray(Wq[:, hs]).astype(bf),
            "wk": np.ascontiguousarray(Wk[:, kvh * D_HEAD:(kvh + 1) * D_HEAD]).astype(bf),
            "wv": np.ascontiguousarray(Wv[:, kvh * D_HEAD:(kvh + 1) * D_HEAD]).astype(bf),
            "wo": np.ascontiguousarray(Wo[hs, :]).astype(bf),
            "wa": pol_WA.astype(bf),
            "wb": pol_WB.astype(bf),
            "cosf": cosf, "sinf": sinf,
            "hpar": np.array([[c_h[2 * c], c_h[2 * c + 1],
                               gate[2 * c], gate[2 * c + 1]]], dtype=np.float32),
        })

    if _NC_CACHE is None:
        _NC_CACHE = _build_kernel()
    from concourse.bass_utils import run_bass_kernel_spmd
    res = run_bass_kernel_spmd(_NC_CACHE, in_maps, core_ids=list(range(N_CORES)))
    total = np.zeros((T, D_MODEL), dtype=np.float32)
    for c in range(N_CORES):
        total += res.results[c]["out"]
    return total[None, :, :]
